# revision 18
# baseline (speedup 1.0000x reference)
"""Trainium2 Bass kernel for the scatter_memory GRU memory-update module.

Computation (torch GRUCell semantics, chunk order r, z, n):
    current = memory[node_ids]                       # [B, H] gather
    gi = messages @ W_ih.T + b_ih ; gh = current @ W_hh.T + b_hh
    r = sigmoid(gi_r + gh_r) ; z = sigmoid(gi_z + gh_z)
    n = tanh(gi_n + r * gh_n)
    updated = (1 - z) * n + z * current
    new_memory = memory.at[node_ids].set(updated)    # scatter
"""

import os
import sys

import numpy as np

for _p in ("/opt/trn_rl_repo", "/root/.axon_site/_ro/trn_rl_repo"):
    if os.path.isdir(_p) and _p not in sys.path:
        sys.path.insert(0, _p)

import ml_dtypes
from contextlib import ExitStack

import concourse.bass as bass
import concourse.tile as tile
from concourse import mybir
from concourse.bass_utils import run_bass_kernel_spmd

BF16 = ml_dtypes.bfloat16
F8 = ml_dtypes.float8_e4m3          # TRN fp8e4: e4m3 with +-240 max
import json as _json

N_CORES = 8
H = 128
NTILE = 1024         # batch columns per PSUM tile (2 banks of fp32 per gate)
DMA_CHUNK = 2048     # batch columns per input DMA

# exposed for test harnesses
LAST_RESULT = None

_NC_CACHE = {}


def _dma_chunks(bpc: int) -> list[tuple[int, int]]:
    """Input DMA schedule: a small first chunk so compute starts early,
    then wide transfers."""
    sizes = [min(512, bpc)]
    pos = sizes[0]
    if pos + 1024 <= bpc:
        sizes.append(1024)
        pos += 1024
    while pos < bpc:
        s = min(DMA_CHUNK, bpc - pos)
        sizes.append(s)
        pos += s
    out = []
    pos = 0
    for s in sizes:
        out.append((pos, s))
        pos += s
    assert pos == bpc
    return out


def _tiles(bpc: int) -> list[tuple[int, int]]:
    """Compute-tile schedule: 1024-wide steady state (PSUM capacity),
    tapered tail so the final serial chain is short.  Tiles never cross
    an input-DMA chunk boundary."""
    out = []
    chunks = _dma_chunks(bpc)
    for ci, (c0, csz) in enumerate(chunks):
        if ci == len(chunks) - 1 and csz > 512:
            pos = c0
            rem = csz
            while rem > 1024:
                out.append((pos, 1024))
                pos += 1024
                rem -= 1024
            for tail in (512, 256, 256):
                if rem <= 0:
                    break
                s = min(tail, rem)
                out.append((pos, s))
                pos += s
                rem -= s
            assert rem == 0, (rem, csz)
        else:
            for p in range(c0, c0 + csz, NTILE):
                out.append((p, min(NTILE, c0 + csz - p)))
    assert sum(s for _, s in out) == bpc
    return out


def _split_sync_waits(bir: dict) -> dict:
    """Hoist extra per-instruction semaphore waits into standalone
    EventSemaphore instructions.

    The walrus build in this container encodes at most ONE sync wait per
    instruction ("Too many sync wait commands" otherwise); Tile attaches
    one wait per dependency.  An engine-level standalone wait immediately
    before the instruction is semantically identical (the engine stalls
    either way), so keep the last wait inline and hoist the rest.
    """
    n = 0
    for fn in bir.get("functions", []):
        for blk in fn.get("blocks", []):
            out = []
            for inst in blk.get("instructions", []):
                si = inst.get("sync_info") or {}
                ow = si.get("on_wait") or []
                if len(ow) > 1:
                    for w in ow[:-1]:
                        n += 1
                        out.append({
                            "debug": inst.get("debug", 0),
                            "engine": inst["engine"],
                            "ins": [],
                            "outs": [],
                            "name": f"hoistw_{n}_{inst['name']}",
                            "opcode": "EventSemaphore",
                            "sync_info": {"on_update": [], "on_wait": [w]},
                        })
                    si["on_wait"] = [ow[-1]]
                out.append(inst)
            blk["instructions"] = out
    return bir


def _patch_json(nc: bass.Bass) -> None:
    orig = nc.to_json_bytes

    def patched() -> bytes:
        return _json.dumps(_split_sync_waits(_json.loads(orig()))).encode()

    nc.to_json_bytes = patched


def _build_nc(bpc: int) -> bass.Bass:
    """Bass program for one core: GRU over a [H, bpc] feature-major shard.

    Per-input-chunk byte layout (uint8 dram tensor `inp`, 4*csz bytes):
        [x_f8 (csz) | h_f8 (csz) | h_bf16 (2*csz)]
    x_f8|h_f8 doubles as the DoubleRow rhs [K, 2, N] (k-tile stride csz).

    Gate matmuls: r/z via fp8 DoubleRow (merges the W_ih@x and W_hh@h
    contractions into one pass at 2 elem/cycle); i_n via plain fp8;
    h_n in bf16 (its path feeds tanh directly, keep it accurate).

    The DVE STT writes t = (h_n + b_hn)*r back INTO the p_hn PSUM bank
    (f32, in-place); the i_n matmuls then accumulate onto it with
    start=False one period later, so `pre = t + i_n` happens on the PE
    for free and tanh reads the finished pre-activation from PSUM.

    Software-pipelined per-engine stream order (period k):
        PE : inMM(k-1), rMM(k), zMM(k), hnMM(k)
        ACT: tanh(k-1), sig_r(k), sig_z(k)
        DVE: m(k-2), out(k-2), d(k-1), STT(k)
    so every instruction's dependencies were issued >= 1 period ago.
    """
    assert bpc % 512 == 0
    f32 = mybir.dt.float32
    bf16 = mybir.dt.bfloat16
    f8e4 = mybir.dt.float8e4
    u8 = mybir.dt.uint8
    sig = mybir.ActivationFunctionType.Sigmoid
    tanh = mybir.ActivationFunctionType.Tanh
    add_op = mybir.AluOpType.add
    mult_op = mybir.AluOpType.mult
    DR = mybir.MatmulPerfMode.DoubleRow

    nc = bass.Bass()
    inp = nc.declare_dram_parameter("inp", [H, 4 * bpc], u8, isOutput=False)
    # packed [w_hhT_n bf16 (2H bytes) | biases bf16 (8) | fp8 ihr|hhr|ihz|hhz|ihn (5H)]
    # bias columns: 0 = b_ih_r + b_hh_r, 1 = b_ih_z + b_hh_z, 2 = b_hh_n, 3 = b_ih_n
    wb = nc.declare_dram_parameter("wb", [H, 7 * H + 8], u8, isOutput=False)
    outT = nc.declare_dram_parameter("outT", [H, bpc], bf16, isOutput=True)

    with ExitStack() as ctx:
        tc = ctx.enter_context(tile.TileContext(nc))
        singles = ctx.enter_context(tc.tile_pool(name="singles", bufs=1))
        io = ctx.enter_context(tc.tile_pool(name="io", bufs=1))
        mids = ctx.enter_context(tc.tile_pool(name="mids", bufs=5))
        outs = ctx.enter_context(tc.tile_pool(name="outs", bufs=4))
        psum = ctx.enter_context(tc.tile_pool(name="psum", bufs=1, space="PSUM"))

        # weights + biases land first via one sync-HWDGE descriptor; they
        # gate the very first matmul / sigmoid
        wb_sb = singles.tile([H, 7 * H + 8], u8)
        nc.sync.dma_start(out=wb_sb, in_=wb[:, :])
        w_hhn_sb = wb_sb[:, 0 : 2 * H].bitcast(bf16)                 # [H, H]
        b_sb = wb_sb[:, 2 * H : 2 * H + 8].bitcast(bf16)             # [H, 4]
        f8w = wb_sb[:, 2 * H + 8 : 7 * H + 8].bitcast(f8e4)          # [H, 5H]
        rz4 = f8w[:, 0 : 4 * H].rearrange("p (four m) -> p four m", four=4)
        lhsT_r = rz4[:, 0:2, :]                                      # [H, 2, H]
        lhsT_z = rz4[:, 2:4, :]
        lhsT_in = f8w[:, 4 * H : 5 * H]                              # [H, H]

        # dummy sigmoid fires the ~2.7us ACT table load immediately, so it
        # overlaps the DMA ramp instead of stalling the first real sigmoid
        # (memset first so it doesn't wait on any DMA)
        warm_sb = singles.tile([H, 1], f32)
        nc.vector.memset(warm_sb, 0.0)
        nc.scalar.activation(out=warm_sb, in_=warm_sb,
                             func=sig, bias=0.0, scale=1.0)

        # Pre-issue EVERY input DMA before any compute/output instruction
        # lands in the sync queue: descriptor generation costs ~0.7us on
        # the issuing engine, and an out-DMA interleaved in the stream
        # would stall all later in-DMA issues behind that chunk's compute.
        # All chunks get distinct tiles (whole input fits in SBUF).
        parts = []
        for ci, (c0, csz) in enumerate(_dma_chunks(bpc)):
            t = io.tile([H, 4 * csz], u8, tag=f"c{ci}")
            nc.sync.dma_start(out=t, in_=inp[:, 4 * c0 : 4 * c0 + 4 * csz])
            parts.append((c0, csz, t))

        def views(lo, n):
            """(xh_f8 [H,2,n], x_f8 [H,n], h_bf [H,n]) for cols [lo,lo+n)."""
            for c0, csz, t in parts:
                if c0 <= lo and lo + n <= c0 + csz:
                    r0 = lo - c0
                    pair = t[:, 0 : 2 * csz].bitcast(f8e4).rearrange(
                        "p (two n) -> p two n", two=2)
                    return (pair[:, :, r0 : r0 + n],
                            t[:, 0:csz].bitcast(f8e4)[:, r0 : r0 + n],
                            t[:, 2 * csz : 4 * csz].bitcast(bf16)[:, r0 : r0 + n])
            raise AssertionError((lo, n))

        tiles = _tiles(bpc)
        n_tiles = len(tiles)
        state = {}

        def stage_front(ti):
            t0, tsz = tiles[ti]
            xh_f8, x_f8, h_bf = views(t0, tsz)

            p_r = psum.tile([H, tsz], f32, tag="p_r")
            p_z = psum.tile([H, tsz], f32, tag="p_z")
            # double-buffered: tile k+1's h_n matmuls need not wait for
            # tile k's tanh to drain the bank (2+2+2x2 = all 8 banks)
            p_hn = psum.tile([H, tsz], f32, tag="p_hn", bufs=2)

            for q0 in range(0, tsz, 512):
                qn = min(512, tsz - q0)
                qs = slice(q0, q0 + qn)
                nc.tensor.matmul(p_r[:, qs], lhsT_r, xh_f8[:, :, qs],
                                 start=True, stop=True, perf_mode=DR)
            for q0 in range(0, tsz, 512):
                qn = min(512, tsz - q0)
                qs = slice(q0, q0 + qn)
                nc.tensor.matmul(p_z[:, qs], lhsT_z, xh_f8[:, :, qs],
                                 start=True, stop=True, perf_mode=DR)
            for q0 in range(0, tsz, 512):
                qs = slice(q0, q0 + min(512, tsz - q0))
                nc.tensor.matmul(p_hn[:, qs], w_hhn_sb, h_bf[:, qs],
                                 start=True, stop=False)

            r_t = mids.tile([H, tsz], bf16, tag="r")
            z_t = mids.tile([H, tsz], bf16, tag="z")
            nc.scalar.activation(out=r_t, in_=p_r, func=sig,
                                 bias=b_sb[:, 0:1], scale=1.0)
            nc.scalar.activation(out=z_t, in_=p_z, func=sig,
                                 bias=b_sb[:, 1:2], scale=1.0)

            # in-place: p_hn <- (p_hn + b_hn) * r   (f32, stays in PSUM)
            nc.vector.scalar_tensor_tensor(
                out=p_hn, in0=p_hn, scalar=b_sb[:, 2:3], in1=r_t,
                op0=add_op, op1=mult_op)
            state[ti] = {"h": h_bf, "x": x_f8, "z": z_t, "p_hn": p_hn}

        def stage_in_mm(ti):
            """i_n matmuls accumulate onto the STT result (one period
            after front; first in the PE stream so tanh unblocks early)."""
            t0, tsz = tiles[ti]
            st = state[ti]
            p_hn, x_f8 = st["p_hn"], st["x"]
            for q0 in range(0, tsz, 512):
                qs = slice(q0, q0 + min(512, tsz - q0))
                nc.tensor.matmul(p_hn[:, qs], lhsT_in, x_f8[:, qs],
                                 start=False, stop=True,
                                 skip_group_check=True)

        def stage_tanh(ti):
            t0, tsz = tiles[ti]
            st = state[ti]
            n_t = mids.tile([H, tsz], bf16, tag="n")
            nc.scalar.activation(out=n_t, in_=st["p_hn"], func=tanh,
                                 bias=b_sb[:, 3:4], scale=1.0)
            st["n"] = n_t

        def stage_d(ti):
            t0, tsz = tiles[ti]
            st = state[ti]
            d_t = mids.tile([H, tsz], bf16, tag="d")
            nc.vector.tensor_sub(out=d_t, in0=st["h"], in1=st["n"])
            st["d"] = d_t

        def stage_blend(ti):
            """m = z*d, out = n + m, DMA (two periods after front)."""
            t0, tsz = tiles[ti]
            st = state.pop(ti)
            m_t = mids.tile([H, tsz], bf16, tag="m")
            o_t = outs.tile([H, tsz], bf16, tag="o")
            nc.vector.tensor_mul(out=m_t, in0=st["z"], in1=st["d"])
            nc.vector.tensor_add(out=o_t, in0=st["n"], in1=m_t)
            nc.sync.dma_start(out=outT[:, t0 : t0 + tsz], in_=o_t)

        for ti in range(n_tiles):
            if ti >= 1:
                stage_in_mm(ti - 1)
                stage_tanh(ti - 1)
            if ti >= 2:
                stage_blend(ti - 2)
            if ti >= 1:
                stage_d(ti - 1)
            stage_front(ti)
        stage_in_mm(n_tiles - 1)
        stage_tanh(n_tiles - 1)
        stage_blend(n_tiles - 2)
        stage_d(n_tiles - 1)
        stage_blend(n_tiles - 1)

    _patch_json(nc)
    return nc


def _get_nc(bpc: int) -> bass.Bass:
    if bpc not in _NC_CACHE:
        _NC_CACHE[bpc] = _build_nc(bpc)
    return _NC_CACHE[bpc]


def kernel(node_ids, messages, memory, W_ih, W_hh, b_ih, b_hh):
    global LAST_RESULT
    node_ids = np.asarray(node_ids)
    messages = np.asarray(messages, dtype=np.float32)
    memory = np.asarray(memory, dtype=np.float32)
    W_ih = np.asarray(W_ih, dtype=np.float32)
    W_hh = np.asarray(W_hh, dtype=np.float32)
    b_ih = np.asarray(b_ih, dtype=np.float32)
    b_hh = np.asarray(b_hh, dtype=np.float32)

    B = node_ids.shape[0]
    per = -(-B // N_CORES)                       # rows per core (unpadded)
    bpc = -(-per // 512) * 512                   # padded to 512 multiple
    nc = _get_nc(bpc)
    chunks = _dma_chunks(bpc)

    current = memory[node_ids]                   # [B, H] host gather

    # weights: [w_hhT_n bf16 | biases bf16 | fp8 ihr|hhr|ihz|hhz|ihn]
    w_ihT = np.ascontiguousarray(W_ih.T)         # [H, 3H]
    w_hhT = np.ascontiguousarray(W_hh.T)
    wb = np.empty((H, 7 * H + 8), dtype=np.uint8)
    wb[:, 0 : 2 * H] = np.ascontiguousarray(
        w_hhT[:, 2 * H : 3 * H].astype(BF16)).view(np.uint8)
    bias = np.empty((H, 4), dtype=np.float32)
    bias[:, 0] = b_ih[0:H] + b_hh[0:H]
    bias[:, 1] = b_ih[H : 2 * H] + b_hh[H : 2 * H]
    bias[:, 2] = b_hh[2 * H : 3 * H]
    bias[:, 3] = b_ih[2 * H : 3 * H]
    wb[:, 2 * H : 2 * H + 8] = bias.astype(BF16).view(np.uint8)
    f8w = np.empty((H, 5 * H), dtype=F8)
    f8w[:, 0 * H : 1 * H] = w_ihT[:, 0:H].astype(F8)
    f8w[:, 1 * H : 2 * H] = w_hhT[:, 0:H].astype(F8)
    f8w[:, 2 * H : 3 * H] = w_ihT[:, H : 2 * H].astype(F8)
    f8w[:, 3 * H : 4 * H] = w_hhT[:, H : 2 * H].astype(F8)
    f8w[:, 4 * H : 5 * H] = w_ihT[:, 2 * H : 3 * H].astype(F8)
    wb[:, 2 * H + 8 :] = f8w.view(np.uint8)

    in_maps = []
    for c in range(N_CORES):
        lo = c * per
        hi = min(lo + per, B)
        xT = np.zeros((H, bpc), dtype=np.float32)
        hT = np.zeros((H, bpc), dtype=np.float32)
        if hi > lo:
            xT[:, : hi - lo] = messages[lo:hi].T
            hT[:, : hi - lo] = current[lo:hi].T
        x_f8 = xT.astype(F8)
        h_f8 = hT.astype(F8)
        h_bf = hT.astype(BF16)
        inp = np.empty((H, 4 * bpc), dtype=np.uint8)
        for c0, csz in chunks:
            o = 4 * c0
            inp[:, o : o + csz] = x_f8[:, c0 : c0 + csz].view(np.uint8)
            inp[:, o + csz : o + 2 * csz] = h_f8[:, c0 : c0 + csz].view(np.uint8)
            inp[:, o + 2 * csz : o + 4 * csz] = np.ascontiguousarray(
                h_bf[:, c0 : c0 + csz]).view(np.uint8)
        in_maps.append({"inp": inp, "wb": wb})

    res = run_bass_kernel_spmd(nc, in_maps, list(range(N_CORES)))
    LAST_RESULT = res

    updated = np.empty((B, H), dtype=np.float32)
    for c in range(N_CORES):
        lo = c * per
        hi = min(lo + per, B)
        if hi > lo:
            updated[lo:hi] = res.results[c]["outT"][:, : hi - lo].T.astype(np.float32)

    new_memory = memory.copy()
    new_memory[node_ids] = updated
    return new_memory


# revision 19
# speedup vs baseline: 1.1604x; 1.1604x over previous
"""Trainium2 Bass kernel for the scatter_memory GRU memory-update module.

Computation (torch GRUCell semantics, chunk order r, z, n):
    current = memory[node_ids]                       # [B, H] gather
    gi = messages @ W_ih.T + b_ih ; gh = current @ W_hh.T + b_hh
    r = sigmoid(gi_r + gh_r) ; z = sigmoid(gi_z + gh_z)
    n = tanh(gi_n + r * gh_n)
    updated = (1 - z) * n + z * current
    new_memory = memory.at[node_ids].set(updated)    # scatter
"""

import os
import sys

import numpy as np

for _p in ("/opt/trn_rl_repo", "/root/.axon_site/_ro/trn_rl_repo"):
    if os.path.isdir(_p) and _p not in sys.path:
        sys.path.insert(0, _p)

import ml_dtypes
from contextlib import ExitStack

import concourse.bass as bass
import concourse.tile as tile
from concourse import mybir
from concourse.bass_utils import run_bass_kernel_spmd

BF16 = ml_dtypes.bfloat16
F8 = ml_dtypes.float8_e4m3          # TRN fp8e4: e4m3 with +-240 max
import json as _json

N_CORES = 8
H = 128
NTILE = 1024         # batch columns per PSUM tile (2 banks of fp32 per gate)
DMA_CHUNK = 2048     # batch columns per input DMA

# exposed for test harnesses
LAST_RESULT = None

_NC_CACHE = {}


def _dma_chunks(bpc: int) -> list[tuple[int, int]]:
    """Input DMA schedule: a small first chunk so compute starts early,
    then wide transfers."""
    sizes = [min(512, bpc)]
    pos = sizes[0]
    if pos + 1024 <= bpc:
        sizes.append(1024)
        pos += 1024
    while pos < bpc:
        s = min(DMA_CHUNK, bpc - pos)
        sizes.append(s)
        pos += s
    out = []
    pos = 0
    for s in sizes:
        out.append((pos, s))
        pos += s
    assert pos == bpc
    return out


def _tiles(bpc: int) -> list[tuple[int, int]]:
    """Compute-tile schedule: 1024-wide steady state (PSUM capacity),
    tapered tail so the final serial chain is short.  Tiles never cross
    an input-DMA chunk boundary."""
    out = []
    chunks = _dma_chunks(bpc)
    for ci, (c0, csz) in enumerate(chunks):
        if ci == len(chunks) - 1 and csz > 512:
            pos = c0
            rem = csz
            while rem > 1024:
                out.append((pos, 1024))
                pos += 1024
                rem -= 1024
            for tail in (512, 256, 256):
                if rem <= 0:
                    break
                s = min(tail, rem)
                out.append((pos, s))
                pos += s
                rem -= s
            assert rem == 0, (rem, csz)
        else:
            for p in range(c0, c0 + csz, NTILE):
                out.append((p, min(NTILE, c0 + csz - p)))
    assert sum(s for _, s in out) == bpc
    return out


def _split_sync_waits(bir: dict) -> dict:
    """Hoist extra per-instruction semaphore waits into standalone
    EventSemaphore instructions.

    The walrus build in this container encodes at most ONE sync wait per
    instruction ("Too many sync wait commands" otherwise); Tile attaches
    one wait per dependency.  An engine-level standalone wait immediately
    before the instruction is semantically identical (the engine stalls
    either way), so keep the last wait inline and hoist the rest.
    """
    n = 0
    for fn in bir.get("functions", []):
        for blk in fn.get("blocks", []):
            out = []
            for inst in blk.get("instructions", []):
                si = inst.get("sync_info") or {}
                ow = si.get("on_wait") or []
                if len(ow) > 1:
                    for w in ow[:-1]:
                        n += 1
                        out.append({
                            "debug": inst.get("debug", 0),
                            "engine": inst["engine"],
                            "ins": [],
                            "outs": [],
                            "name": f"hoistw_{n}_{inst['name']}",
                            "opcode": "EventSemaphore",
                            "sync_info": {"on_update": [], "on_wait": [w]},
                        })
                    si["on_wait"] = [ow[-1]]
                out.append(inst)
            blk["instructions"] = out
    return bir


def _patch_json(nc: bass.Bass) -> None:
    orig = nc.to_json_bytes

    def patched() -> bytes:
        return _json.dumps(_split_sync_waits(_json.loads(orig()))).encode()

    nc.to_json_bytes = patched


def _build_nc(bpc: int) -> bass.Bass:
    """Bass program for one core: GRU over a [H, bpc] feature-major shard.

    Per-input-chunk byte layout (uint8 dram tensor `inp`, 4*csz bytes):
        [x_f8 (csz) | h_f8 (csz) | h_bf16 (2*csz)]
    x_f8|h_f8 doubles as the DoubleRow rhs [K, 2, N] (k-tile stride csz).

    Gate matmuls: r/z via fp8 DoubleRow (merges the W_ih@x and W_hh@h
    contractions into one pass at 2 elem/cycle); i_n via plain fp8;
    h_n in bf16 (its path feeds tanh directly, keep it accurate).

    The DVE STT writes t = (h_n + b_hn)*r back INTO the p_hn PSUM bank
    (f32, in-place); the i_n matmuls then accumulate onto it with
    start=False one period later, so `pre = t + i_n` happens on the PE
    for free and tanh reads the finished pre-activation from PSUM.

    Software-pipelined per-engine stream order (period k):
        PE : inMM(k-1), rMM(k), zMM(k), hnMM(k)
        ACT: tanh(k-1), sig_r(k), sig_z(k)
        DVE: m(k-2), out(k-2), d(k-1), STT(k)
    so every instruction's dependencies were issued >= 1 period ago.
    """
    assert bpc % 512 == 0
    f32 = mybir.dt.float32
    bf16 = mybir.dt.bfloat16
    f8e4 = mybir.dt.float8e4
    u8 = mybir.dt.uint8
    sig = mybir.ActivationFunctionType.Sigmoid
    tanh = mybir.ActivationFunctionType.Tanh
    add_op = mybir.AluOpType.add
    mult_op = mybir.AluOpType.mult
    DR = mybir.MatmulPerfMode.DoubleRow

    nc = bass.Bass()
    inp = nc.declare_dram_parameter("inp", [H, 4 * bpc], u8, isOutput=False)
    # packed [w_ihT | w_hhT | biases], all bf16 bytes
    # bias columns: 0 = b_ih_r + b_hh_r, 1 = b_ih_z + b_hh_z, 2 = b_hh_n, 3 = b_ih_n
    wb = nc.declare_dram_parameter("wb", [H, 12 * H + 8], u8, isOutput=False)
    outT = nc.declare_dram_parameter("outT", [H, bpc], bf16, isOutput=True)

    with ExitStack() as ctx:
        tc = ctx.enter_context(tile.TileContext(nc))
        singles = ctx.enter_context(tc.tile_pool(name="singles", bufs=1))
        io = ctx.enter_context(tc.tile_pool(name="io", bufs=1))
        mids = ctx.enter_context(tc.tile_pool(name="mids", bufs=5))
        outs = ctx.enter_context(tc.tile_pool(name="outs", bufs=4))
        psum = ctx.enter_context(tc.tile_pool(name="psum", bufs=1, space="PSUM"))

        # weights + biases land first via one sync-HWDGE descriptor; they
        # gate the very first matmul / sigmoid
        wb_sb = singles.tile([H, 12 * H + 8], u8)
        nc.sync.dma_start(out=wb_sb, in_=wb[:, :])
        wb_bf = wb_sb[:, :].bitcast(bf16)                            # [H, 6H+4]
        w_ih_sb = wb_bf[:, 0 : 3 * H]
        w_hh_sb = wb_bf[:, 3 * H : 6 * H]
        b_sb = wb_bf[:, 6 * H : 6 * H + 4]

        # dummy sigmoid fires the ~2.7us ACT table load immediately, so it
        # overlaps the DMA ramp instead of stalling the first real sigmoid
        # (memset first so it doesn't wait on any DMA)
        warm_sb = singles.tile([H, 1], f32)
        nc.vector.memset(warm_sb, 0.0)
        nc.scalar.activation(out=warm_sb, in_=warm_sb,
                             func=sig, bias=0.0, scale=1.0)

        # Pre-issue EVERY input DMA before any compute/output instruction
        # lands in the sync queue: descriptor generation costs ~0.7us on
        # the issuing engine, and an out-DMA interleaved in the stream
        # would stall all later in-DMA issues behind that chunk's compute.
        # All chunks get distinct tiles (whole input fits in SBUF).
        parts = []
        for ci, (c0, csz) in enumerate(_dma_chunks(bpc)):
            t = io.tile([H, 4 * csz], u8, tag=f"c{ci}")
            nc.sync.dma_start(out=t, in_=inp[:, 4 * c0 : 4 * c0 + 4 * csz])
            parts.append((c0, csz, t))

        def views(lo, n):
            """(x_bf [H,n], h_bf [H,n]) for cols [lo, lo+n)."""
            for c0, csz, t in parts:
                if c0 <= lo and lo + n <= c0 + csz:
                    r0 = lo - c0
                    return (t[:, 0 : 2 * csz].bitcast(bf16)[:, r0 : r0 + n],
                            t[:, 2 * csz : 4 * csz].bitcast(bf16)[:, r0 : r0 + n])
            raise AssertionError((lo, n))

        tiles = _tiles(bpc)
        n_tiles = len(tiles)
        state = {}

        def stage_front(ti):
            t0, tsz = tiles[ti]
            x_sb, h_sb = views(t0, tsz)

            p_r = psum.tile([H, tsz], f32, tag="p_r")
            p_z = psum.tile([H, tsz], f32, tag="p_z")
            # double-buffered: tile k+1's h_n matmuls need not wait for
            # tile k's tanh to drain the bank (2+2+2x2 = all 8 banks)
            p_hn = psum.tile([H, tsz], f32, tag="p_hn", bufs=2)

            for q0 in range(0, tsz, 512):
                qs = slice(q0, q0 + min(512, tsz - q0))
                nc.tensor.matmul(p_r[:, qs], w_ih_sb[:, 0:H], x_sb[:, qs],
                                 start=True, stop=False)
                nc.tensor.matmul(p_r[:, qs], w_hh_sb[:, 0:H], h_sb[:, qs],
                                 start=False, stop=True)
            for q0 in range(0, tsz, 512):
                qs = slice(q0, q0 + min(512, tsz - q0))
                nc.tensor.matmul(p_z[:, qs], w_ih_sb[:, H : 2 * H],
                                 x_sb[:, qs], start=True, stop=False)
                nc.tensor.matmul(p_z[:, qs], w_hh_sb[:, H : 2 * H],
                                 h_sb[:, qs], start=False, stop=True)
            for q0 in range(0, tsz, 512):
                qs = slice(q0, q0 + min(512, tsz - q0))
                nc.tensor.matmul(p_hn[:, qs], w_hh_sb[:, 2 * H : 3 * H],
                                 h_sb[:, qs], start=True, stop=False)

            r_t = mids.tile([H, tsz], bf16, tag="r")
            z_t = mids.tile([H, tsz], bf16, tag="z")
            nc.scalar.activation(out=r_t, in_=p_r, func=sig,
                                 bias=b_sb[:, 0:1], scale=1.0)
            nc.scalar.activation(out=z_t, in_=p_z, func=sig,
                                 bias=b_sb[:, 1:2], scale=1.0)

            # in-place: p_hn <- (p_hn + b_hn) * r   (f32, stays in PSUM)
            nc.vector.scalar_tensor_tensor(
                out=p_hn, in0=p_hn, scalar=b_sb[:, 2:3], in1=r_t,
                op0=add_op, op1=mult_op)
            state[ti] = {"h": h_sb, "x": x_sb, "z": z_t, "p_hn": p_hn}

        def stage_in_mm(ti):
            """i_n matmuls accumulate onto the STT result (one period
            after front; first in the PE stream so tanh unblocks early)."""
            t0, tsz = tiles[ti]
            st = state[ti]
            p_hn, x_sb = st["p_hn"], st["x"]
            for q0 in range(0, tsz, 512):
                qs = slice(q0, q0 + min(512, tsz - q0))
                nc.tensor.matmul(p_hn[:, qs], w_ih_sb[:, 2 * H : 3 * H],
                                 x_sb[:, qs], start=False, stop=True,
                                 skip_group_check=True)

        def stage_tanh(ti):
            t0, tsz = tiles[ti]
            st = state[ti]
            n_t = mids.tile([H, tsz], bf16, tag="n")
            nc.scalar.activation(out=n_t, in_=st["p_hn"], func=tanh,
                                 bias=b_sb[:, 3:4], scale=1.0)
            st["n"] = n_t

        def stage_d(ti):
            t0, tsz = tiles[ti]
            st = state[ti]
            d_t = mids.tile([H, tsz], bf16, tag="d")
            nc.vector.tensor_sub(out=d_t, in0=st["h"], in1=st["n"])
            st["d"] = d_t

        def stage_blend(ti):
            """m = z*d, out = n + m, DMA (two periods after front)."""
            t0, tsz = tiles[ti]
            st = state.pop(ti)
            m_t = mids.tile([H, tsz], bf16, tag="m")
            o_t = outs.tile([H, tsz], bf16, tag="o")
            nc.vector.tensor_mul(out=m_t, in0=st["z"], in1=st["d"])
            nc.vector.tensor_add(out=o_t, in0=st["n"], in1=m_t)
            nc.sync.dma_start(out=outT[:, t0 : t0 + tsz], in_=o_t)

        for ti in range(n_tiles):
            if ti >= 1:
                stage_in_mm(ti - 1)
                stage_tanh(ti - 1)
            if ti >= 2:
                stage_blend(ti - 2)
            if ti >= 1:
                stage_d(ti - 1)
            stage_front(ti)
        stage_in_mm(n_tiles - 1)
        stage_tanh(n_tiles - 1)
        stage_blend(n_tiles - 2)
        stage_d(n_tiles - 1)
        stage_blend(n_tiles - 1)

    _patch_json(nc)
    return nc


def _get_nc(bpc: int) -> bass.Bass:
    if bpc not in _NC_CACHE:
        _NC_CACHE[bpc] = _build_nc(bpc)
    return _NC_CACHE[bpc]


def kernel(node_ids, messages, memory, W_ih, W_hh, b_ih, b_hh):
    global LAST_RESULT
    node_ids = np.asarray(node_ids)
    messages = np.asarray(messages, dtype=np.float32)
    memory = np.asarray(memory, dtype=np.float32)
    W_ih = np.asarray(W_ih, dtype=np.float32)
    W_hh = np.asarray(W_hh, dtype=np.float32)
    b_ih = np.asarray(b_ih, dtype=np.float32)
    b_hh = np.asarray(b_hh, dtype=np.float32)

    B = node_ids.shape[0]
    per = -(-B // N_CORES)                       # rows per core (unpadded)
    bpc = -(-per // 512) * 512                   # padded to 512 multiple
    nc = _get_nc(bpc)
    chunks = _dma_chunks(bpc)

    current = memory[node_ids]                   # [B, H] host gather

    # weights: [w_ihT | w_hhT | biases], all bf16
    wbf = np.empty((H, 6 * H + 4), dtype=np.float32)
    wbf[:, 0 : 3 * H] = W_ih.T
    wbf[:, 3 * H : 6 * H] = W_hh.T
    wbf[:, 6 * H + 0] = b_ih[0:H] + b_hh[0:H]
    wbf[:, 6 * H + 1] = b_ih[H : 2 * H] + b_hh[H : 2 * H]
    wbf[:, 6 * H + 2] = b_hh[2 * H : 3 * H]
    wbf[:, 6 * H + 3] = b_ih[2 * H : 3 * H]
    wb = wbf.astype(BF16).view(np.uint8)

    in_maps = []
    for c in range(N_CORES):
        lo = c * per
        hi = min(lo + per, B)
        xT = np.zeros((H, bpc), dtype=np.float32)
        hT = np.zeros((H, bpc), dtype=np.float32)
        if hi > lo:
            xT[:, : hi - lo] = messages[lo:hi].T
            hT[:, : hi - lo] = current[lo:hi].T
        x_bf = xT.astype(BF16)
        h_bf = hT.astype(BF16)
        inp = np.empty((H, 4 * bpc), dtype=np.uint8)
        for c0, csz in chunks:
            o = 4 * c0
            inp[:, o : o + 2 * csz] = np.ascontiguousarray(
                x_bf[:, c0 : c0 + csz]).view(np.uint8)
            inp[:, o + 2 * csz : o + 4 * csz] = np.ascontiguousarray(
                h_bf[:, c0 : c0 + csz]).view(np.uint8)
        in_maps.append({"inp": inp, "wb": wb})

    res = run_bass_kernel_spmd(nc, in_maps, list(range(N_CORES)))
    LAST_RESULT = res

    updated = np.empty((B, H), dtype=np.float32)
    for c in range(N_CORES):
        lo = c * per
        hi = min(lo + per, B)
        if hi > lo:
            updated[lo:hi] = res.results[c]["outT"][:, : hi - lo].T.astype(np.float32)

    new_memory = memory.copy()
    new_memory[node_ids] = updated
    return new_memory
